# revision 13
# baseline (speedup 1.0000x reference)
"""LINK forward (gnn message passing SpMM) on 8 TRN2 NeuronCores.

out[r, :] = W_bias + sum_{e: row[e]==r} W_weight.T[col[e], :]

Design (per core, SPMD over 8 cores; host shards rows and reassembles):
  - Global 128-row tiles (782) are load-balanced across the 8 cores by
    sorted block count, so the SPMD shared schedule has near-zero
    max-over-cores padding.
  - Edges within a tile are sorted by col in alternating (zigzag)
    direction per tile slot, so every 1024-idx dma_gather call covers a
    <32768-wide col window with a per-call wt base: int16 indices with
    no 4-chunk ceil padding and all calls full-size (1024 = the 16KB
    per-engine packet cap of the SWDGE gather).
  - One-hot S per block (DVE is_equal vs iota), per-tile PSUM
    accumulation via S^T @ G matmuls, bias fused in the PSUM->SBUF
    copy, output DMA'd per tile.
  - Small idx head DMA'd first so the gather pipeline starts ~3us in;
    the 6.8MB idx tail streams behind the first 16 calls.
  - num_idxs register shared across calls (one MOVE total).

Perf note: the kernel is bound by SWDGE descriptor generation on the
Pool engine (~2.3ns/descriptor marginal, measured; ~408k descriptors
per core). DMA transfer (18.8ns/desc across 16 engines), tensor
(37%) and vector (50%) all fit inside that shadow. Probed
alternatives that do NOT beat it: bigger gather calls (16KB packet cap,
ring depth 1/queue), single_packet=False (2x DMA cost), smaller calls
(per-call overhead), SBUF-source gather (transpose-only layout),
tensor-engine one-hot gather (slot x col-page density ~5 edges makes
any 2-stage matmul gather/scatter pay >20x), gpsimd ap_gather/
scatter_add (bf16 accumulation imprecise; Pool-bound anyway).
"""

import sys

sys.path.insert(0, "/opt/trn_rl_repo")

import numpy as np
import ml_dtypes

import concourse.bass as bass
import concourse.tile as tile
from concourse import bacc, mybir
from concourse.bass_utils import run_bass_kernel_spmd

P = 128
D = 128
N = 100000
NCORE = 8
NTG = (N + P - 1) // P         # 782 global tiles (last has 32 rows)
SLOTS = (NTG + NCORE - 1) // NCORE  # 98 tile slots per core
CALL = 1024
NBC = CALL // P                # 8 blocks per full call
WIN = 32768                    # int16 col window

LAST_EXEC_NS = None
_CACHE = {}


def _prepare(edge_index):
    """Returns (nbs [SLOTS] common block counts, tile_map [NCORE][SLOTS],
    calls [(idx_off, n_idx, base, src_rows, blk_off)], idx_arr [NCORE, ltot]
    int16, m_arr [NCORE, ltot] f32)."""
    row = np.asarray(edge_index[0], dtype=np.int64)
    col = np.asarray(edge_index[1], dtype=np.int64)

    tg = row >> 7
    m = row & 127
    cnt = np.bincount(tg, minlength=NTG)                 # [782]
    nb = np.maximum(1, -(-cnt // P))                     # blocks per tile

    # snake-assign tiles (sorted by nb desc) to NCORE x SLOTS grid
    order = np.argsort(-nb, kind="stable")
    tile_map = np.full((NCORE, SLOTS), -1, np.int64)
    for j in range(SLOTS):
        chunk = order[j * NCORE:(j + 1) * NCORE]
        cores = range(NCORE) if j % 2 == 0 else range(NCORE - 1, -1, -1)
        for ci, c in enumerate(cores):
            if ci < len(chunk):
                tile_map[c, j] = chunk[ci]
    nbs = np.zeros(SLOTS, np.int64)
    for j in range(SLOTS):
        ts = tile_map[:, j]
        nbs[j] = max(int(nb[t]) if t >= 0 else 1 for t in ts)
    blk_start = np.zeros(SLOTS, np.int64)
    blk_start[1:] = np.cumsum(nbs)[:-1]
    nblk_tot = int(nbs.sum())
    ltot = nblk_tot * P

    # per-core streams: cols (zigzag-sorted) + m, padded per slot
    colstr = np.zeros((NCORE, ltot), np.int64)   # absolute col per slot
    validm = np.zeros((NCORE, ltot), bool)
    m_arr = np.full((NCORE, ltot), 200.0, np.float32)

    order_e = np.argsort(tg, kind="stable")
    tg_s = tg[order_e]
    col_s = col[order_e]
    m_s = m[order_e]
    tstart = np.zeros(NTG + 1, np.int64)
    tstart[1:] = np.cumsum(cnt)

    for c in range(NCORE):
        for j in range(SLOTS):
            t = int(tile_map[c, j])
            off = int(blk_start[j]) * P
            if t < 0:
                continue
            a, b = int(tstart[t]), int(tstart[t + 1])
            cc = col_s[a:b]
            mm = m_s[a:b]
            o = np.argsort(cc, kind="stable")
            if j % 2 == 1:
                o = o[::-1]
            n = b - a
            if n == 0:
                continue
            # stretch the sorted stream across the slot's common length so
            # every core's col progression aligns per block
            L = int(nbs[j]) * P
            pos = off + np.arange(n, dtype=np.int64) * L // n
            colstr[c, pos] = cc[o]
            m_arr[c, pos] = mm[o]
            validm[c, pos] = True

    # call splitting: walk blocks, cut at CALL blocks or window overflow
    calls = []
    idx_arr = np.zeros((NCORE, ltot), np.int16)
    b0 = 0
    while b0 < nblk_tot:
        be = min(b0 + NBC, nblk_tot)
        while True:
            lo, hi = N, 0
            s0, s1 = b0 * P, be * P
            for c in range(NCORE):
                v = validm[c, s0:s1]
                if v.any():
                    cs = colstr[c, s0:s1][v]
                    lo = min(lo, int(cs.min()))
                    hi = max(hi, int(cs.max()))
            if hi < lo:
                lo = 0
                break
            if hi - lo < WIN:
                break
            be -= 1
            assert be > b0, "single block exceeds col window"
        base = lo
        src_rows = min(WIN, N - base)
        n_idx = (be - b0) * P
        for c in range(NCORE):
            s0, s1 = b0 * P, be * P
            rel = colstr[c, s0:s1] - base
            rel[~validm[c, s0:s1]] = 0
            assert rel.min() >= 0 and rel.max() < src_rows
            idx_arr[c, s0:s1] = rel.astype(np.int16)
        calls.append((b0 * P, n_idx, base, src_rows, b0))
        b0 = be

    blk2call = np.zeros(nblk_tot, np.int64)
    for ci, (_, n_idx, _, _, blk_off) in enumerate(calls):
        blk2call[blk_off:blk_off + n_idx // P] = ci

    return nbs, tile_map, calls, blk2call, idx_arr, m_arr


def _build(nbs, calls, blk2call):
    ltot = int(nbs.sum()) * P
    nc = bacc.Bacc("TRN2", target_bir_lowering=False, num_swdge_queues=4)
    wt = nc.dram_tensor("wt", [N, D], mybir.dt.bfloat16, kind="ExternalInput")
    bias = nc.dram_tensor("bias", [P, D], mybir.dt.float32, kind="ExternalInput")
    SPL = 1024  # idx cols in the small head piece (16 calls)
    idx_da = nc.dram_tensor("idxa", [P, SPL], mybir.dt.int16,
                            kind="ExternalInput")
    idx_db = nc.dram_tensor("idxb", [P, ltot // 16 - SPL], mybir.dt.int16,
                            kind="ExternalInput")
    m_d = nc.dram_tensor("m", [P, ltot // P], mybir.dt.bfloat16,
                         kind="ExternalInput")
    out = nc.dram_tensor("out", [SLOTS * P, D], mybir.dt.float32,
                         kind="ExternalOutput")

    with tile.TileContext(nc) as tc:
        with tc.tile_pool(name="const", bufs=1) as cpool, \
             tc.tile_pool(name="g", bufs=16) as gpool, \
             tc.tile_pool(name="s", bufs=16) as spool, \
             tc.tile_pool(name="o", bufs=6) as opool, \
             tc.tile_pool(name="psum", bufs=7, space="PSUM") as pspool:

            # small idx head + m first so the gather pipeline starts ASAP;
            # the big idx_b tail streams in behind the first 16 calls
            idx_a = cpool.tile([P, 1024], mybir.dt.int16)
            nc.sync.dma_start(idx_a[:], idx_da[:])
            m_all = cpool.tile([P, ltot // P], mybir.dt.bfloat16)
            nc.sync.dma_start(m_all[:], m_d[:])
            bias_t = cpool.tile([P, D], mybir.dt.float32)
            nc.sync.dma_start(bias_t[:], bias[:])
            idx_b = cpool.tile([P, ltot // 16 - 1024], mybir.dt.int16)
            nc.sync.dma_start(idx_b[:], idx_db[:])
            iota16 = cpool.tile([P, P], mybir.dt.int16)
            nc.gpsimd.iota(iota16[:], pattern=[[1, P]], base=0, channel_multiplier=0)
            iota_bf = cpool.tile([P, P], mybir.dt.bfloat16)
            nc.vector.tensor_copy(iota_bf[:], iota16[:])

            gq = [0]
            g_tiles = {}
            s_tiles = {}
            # one shared register per distinct num_idxs (saves a gpsimd MOVE
            # per gather call)
            nreg_cache = {}

            def nreg(n):
                if n not in nreg_cache:
                    nreg_cache[n] = nc.gpsimd.to_reg(n)
                return nreg_cache[n]

            def ensure(ci):
                if ci in g_tiles:
                    return
                idx_off, n_idx, base, src_rows, blk_off = calls[ci]
                nb_s = n_idx // P
                g = gpool.tile([P, NBC, D], mybir.dt.bfloat16, tag="g")
                c0 = idx_off // 16
                if c0 + n_idx // 16 <= 1024:
                    islice = idx_a[:, c0: c0 + n_idx // 16]
                else:
                    islice = idx_b[:, c0 - 1024: c0 - 1024 + n_idx // 16]
                nc.gpsimd.dma_gather(
                    g[:, :nb_s, :],
                    wt[base:base + src_rows, :],
                    islice,
                    n_idx, nreg(n_idx), D, queue_num=gq[0] % 4,
                )
                gq[0] += 1
                st = spool.tile([P, NBC * P], mybir.dt.bfloat16, tag="s")
                nc.vector.tensor_tensor(
                    out=st[:, :n_idx].rearrange("p (b m) -> p b m", m=P),
                    in0=m_all[:, blk_off: blk_off + nb_s].unsqueeze(2)
                        .broadcast_to([P, nb_s, P]),
                    in1=iota_bf[:].unsqueeze(1).broadcast_to([P, nb_s, P]),
                    op=mybir.AluOpType.is_equal,
                )
                g_tiles[ci] = g
                s_tiles[ci] = st

            b = 0
            for j in range(SLOTS):
                nbj = int(nbs[j])
                ps = pspool.tile([P, D], mybir.dt.float32, space="PSUM")
                for i in range(nbj):
                    ci = int(blk2call[b])
                    b_loc = b - calls[ci][4]
                    ensure(ci)
                    nc.tensor.matmul(
                        out=ps[:],
                        lhsT=s_tiles[ci][:, b_loc * P:(b_loc + 1) * P],
                        rhs=g_tiles[ci][:, b_loc, :],
                        start=(i == 0),
                        stop=(i == nbj - 1),
                    )
                    b += 1
                ot = opool.tile([P, D], mybir.dt.float32, tag="o")
                nc.vector.tensor_tensor(
                    out=ot[:], in0=ps[:], in1=bias_t[:],
                    op=mybir.AluOpType.add,
                )
                nc.sync.dma_start(out[j * P:(j + 1) * P, :], ot[:])
    nc.compile()
    return nc


def kernel(x=None, edge_index=None, W_weight=None, W_bias=None, _trace=False):
    global LAST_EXEC_NS
    edge_index = np.asarray(edge_index)
    W_weight = np.asarray(W_weight, dtype=np.float32)
    W_bias = np.asarray(W_bias, dtype=np.float32)

    key = (edge_index.tobytes()[:4096], edge_index.shape)
    cached = _CACHE.get(key)
    if cached is None:
        nbs, tile_map, calls, blk2call, idx_arr, m_arr = _prepare(edge_index)
        nc = _build(nbs, calls, blk2call)
        wt_bf = np.ascontiguousarray(W_weight.T).astype(ml_dtypes.bfloat16)
        bias_b = np.tile(W_bias[None, :], (P, 1)).astype(np.float32)
        in_maps = []
        for c in range(NCORE):
            im = {
                "wt": wt_bf,
                "bias": bias_b,
                "idxa": np.ascontiguousarray(
                    np.tile(idx_arr[c].reshape(-1, 16).T, (8, 1))[:, :1024]
                ).astype(np.int16),
                "idxb": np.ascontiguousarray(
                    np.tile(idx_arr[c].reshape(-1, 16).T, (8, 1))[:, 1024:]
                ).astype(np.int16),
                "m": np.ascontiguousarray(
                    m_arr[c].reshape(-1, P).T).astype(ml_dtypes.bfloat16),
            }
            in_maps.append(im)
        _CACHE[key] = (nc, in_maps, tile_map)
    else:
        nc, in_maps, tile_map = cached

    res = run_bass_kernel_spmd(nc, in_maps, core_ids=list(range(NCORE)),
                               trace=_trace)
    LAST_EXEC_NS = res.exec_time_ns
    outp = np.zeros((N, D), np.float32)
    for c in range(NCORE):
        r = res.results[c]["out"].astype(np.float32)
        for j in range(SLOTS):
            t = int(tile_map[c, j])
            if t < 0:
                continue
            rows = min(P, N - t * P)
            outp[t * P: t * P + rows] = r[j * P: j * P + rows]
    return outp



# revision 15
# speedup vs baseline: 1.1374x; 1.1374x over previous
"""LINK forward (gnn message passing SpMM) on 8 TRN2 NeuronCores.

out[r, :] = W_bias + sum_{e: row[e]==r} W_weight.T[col[e], :]

Design (per core, SPMD over 8 cores; host shards rows and reassembles):
  - Global 128-row tiles (782) are load-balanced across the 8 cores by
    sorted block count, so the SPMD shared schedule has near-zero
    max-over-cores padding.
  - Edges within a tile are sorted by col in alternating (zigzag)
    direction per tile slot, so every 1024-idx dma_gather call covers a
    <32768-wide col window with a per-call wt base: int16 indices with
    no 4-chunk ceil padding and all calls full-size (1024 = the 16KB
    per-engine packet cap of the SWDGE gather).
  - One-hot S per block (DVE is_equal vs iota), per-tile PSUM
    accumulation via S^T @ G matmuls, bias fused in the PSUM->SBUF
    copy, output DMA'd per tile.
  - Inputs (idx head/tail, m, bias) are DMA'd fully BEFORE the first
    gather (~21us head): overlapping them with early gathers was
    measured SLOWER (+130us) because the 1KB-packet input transfers
    contend with 16KB gather packets on the 16 DMA engines.

Perf note: the kernel is bound by SWDGE descriptor generation on the
Pool engine (~2.3ns/descriptor marginal, measured; ~408k descriptors
per core). DMA transfer (18.8ns/desc across 16 engines), tensor
(37%) and vector (50%) all fit inside that shadow. Probed
alternatives that do NOT beat it: bigger gather calls (16KB packet cap,
ring depth 1/queue), single_packet=False (2x DMA cost), smaller calls
(per-call overhead), SBUF-source gather (transpose-only layout),
tensor-engine one-hot gather (slot x col-page density ~5 edges makes
any 2-stage matmul gather/scatter pay >20x), gpsimd ap_gather/
scatter_add (bf16 accumulation imprecise; Pool-bound anyway).
"""

import sys

sys.path.insert(0, "/opt/trn_rl_repo")

import numpy as np
import ml_dtypes

import concourse.bass as bass
import concourse.tile as tile
from concourse import bacc, mybir
from concourse.bass_utils import run_bass_kernel_spmd

P = 128
D = 128
N = 100000
NCORE = 8
NTG = (N + P - 1) // P         # 782 global tiles (last has 32 rows)
SLOTS = (NTG + NCORE - 1) // NCORE  # 98 tile slots per core
CALL = 1024
NBC = CALL // P                # 8 blocks per full call
WIN = 32768                    # int16 col window

LAST_EXEC_NS = None
_CACHE = {}


def _prepare(edge_index):
    """Returns (nbs [SLOTS] common block counts, tile_map [NCORE][SLOTS],
    calls [(idx_off, n_idx, base, src_rows, blk_off)], idx_arr [NCORE, ltot]
    int16, m_arr [NCORE, ltot] f32)."""
    row = np.asarray(edge_index[0], dtype=np.int64)
    col = np.asarray(edge_index[1], dtype=np.int64)

    tg = row >> 7
    m = row & 127
    cnt = np.bincount(tg, minlength=NTG)                 # [782]
    nb = np.maximum(1, -(-cnt // P))                     # blocks per tile

    # snake-assign tiles (sorted by nb desc) to NCORE x SLOTS grid
    order = np.argsort(-nb, kind="stable")
    tile_map = np.full((NCORE, SLOTS), -1, np.int64)
    for j in range(SLOTS):
        chunk = order[j * NCORE:(j + 1) * NCORE]
        cores = range(NCORE) if j % 2 == 0 else range(NCORE - 1, -1, -1)
        for ci, c in enumerate(cores):
            if ci < len(chunk):
                tile_map[c, j] = chunk[ci]
    nbs = np.zeros(SLOTS, np.int64)
    for j in range(SLOTS):
        ts = tile_map[:, j]
        nbs[j] = max(int(nb[t]) if t >= 0 else 1 for t in ts)
    blk_start = np.zeros(SLOTS, np.int64)
    blk_start[1:] = np.cumsum(nbs)[:-1]
    nblk_tot = int(nbs.sum())
    ltot = nblk_tot * P

    # per-core streams: cols (zigzag-sorted) + m, padded per slot
    colstr = np.zeros((NCORE, ltot), np.int64)   # absolute col per slot
    validm = np.zeros((NCORE, ltot), bool)
    m_arr = np.full((NCORE, ltot), 200.0, np.float32)

    order_e = np.argsort(tg, kind="stable")
    tg_s = tg[order_e]
    col_s = col[order_e]
    m_s = m[order_e]
    tstart = np.zeros(NTG + 1, np.int64)
    tstart[1:] = np.cumsum(cnt)

    for c in range(NCORE):
        for j in range(SLOTS):
            t = int(tile_map[c, j])
            off = int(blk_start[j]) * P
            if t < 0:
                continue
            a, b = int(tstart[t]), int(tstart[t + 1])
            cc = col_s[a:b]
            mm = m_s[a:b]
            o = np.argsort(cc, kind="stable")
            if j % 2 == 1:
                o = o[::-1]
            n = b - a
            if n == 0:
                continue
            # stretch the sorted stream across the slot's common length so
            # every core's col progression aligns per block
            L = int(nbs[j]) * P
            pos = off + np.arange(n, dtype=np.int64) * L // n
            colstr[c, pos] = cc[o]
            m_arr[c, pos] = mm[o]
            validm[c, pos] = True

    # call splitting: walk blocks, cut at CALL blocks or window overflow
    calls = []
    idx_arr = np.zeros((NCORE, ltot), np.int16)
    b0 = 0
    while b0 < nblk_tot:
        be = min(b0 + NBC, nblk_tot)
        while True:
            lo, hi = N, 0
            s0, s1 = b0 * P, be * P
            for c in range(NCORE):
                v = validm[c, s0:s1]
                if v.any():
                    cs = colstr[c, s0:s1][v]
                    lo = min(lo, int(cs.min()))
                    hi = max(hi, int(cs.max()))
            if hi < lo:
                lo = 0
                break
            if hi - lo < WIN:
                break
            be -= 1
            assert be > b0, "single block exceeds col window"
        base = lo
        src_rows = min(WIN, N - base)
        n_idx = (be - b0) * P
        for c in range(NCORE):
            s0, s1 = b0 * P, be * P
            rel = colstr[c, s0:s1] - base
            rel[~validm[c, s0:s1]] = 0
            assert rel.min() >= 0 and rel.max() < src_rows
            idx_arr[c, s0:s1] = rel.astype(np.int16)
        calls.append((b0 * P, n_idx, base, src_rows, b0))
        b0 = be

    blk2call = np.zeros(nblk_tot, np.int64)
    for ci, (_, n_idx, _, _, blk_off) in enumerate(calls):
        blk2call[blk_off:blk_off + n_idx // P] = ci

    return nbs, tile_map, calls, blk2call, idx_arr, m_arr


def _build(nbs, calls, blk2call):
    ltot = int(nbs.sum()) * P
    nc = bacc.Bacc("TRN2", target_bir_lowering=False, num_swdge_queues=4)
    wt = nc.dram_tensor("wt", [N, D], mybir.dt.bfloat16, kind="ExternalInput")
    bias = nc.dram_tensor("bias", [P, D], mybir.dt.float32, kind="ExternalInput")
    SPL = 1024  # idx cols in the small head piece (16 calls)
    idx_da = nc.dram_tensor("idxa", [P, SPL], mybir.dt.int16,
                            kind="ExternalInput")
    idx_db = nc.dram_tensor("idxb", [P, ltot // 16 - SPL], mybir.dt.int16,
                            kind="ExternalInput")
    m_d = nc.dram_tensor("m", [P, ltot // P], mybir.dt.bfloat16,
                         kind="ExternalInput")
    out = nc.dram_tensor("out", [SLOTS * P, D], mybir.dt.float32,
                         kind="ExternalOutput")

    with tile.TileContext(nc) as tc:
        with tc.tile_pool(name="const", bufs=1) as cpool, \
             tc.tile_pool(name="g", bufs=16) as gpool, \
             tc.tile_pool(name="s", bufs=16) as spool, \
             tc.tile_pool(name="o", bufs=6) as opool, \
             tc.tile_pool(name="psum", bufs=7, space="PSUM") as pspool:

            iota16 = cpool.tile([P, P], mybir.dt.int16)
            nc.gpsimd.iota(iota16[:], pattern=[[1, P]], base=0, channel_multiplier=0)
            iota_bf = cpool.tile([P, P], mybir.dt.bfloat16)
            nc.vector.tensor_copy(iota_bf[:], iota16[:])
            bias_t = cpool.tile([P, D], mybir.dt.float32)
            nc.sync.dma_start(bias_t[:], bias[:])
            idx_a = cpool.tile([P, 1024], mybir.dt.int16)
            nc.sync.dma_start(idx_a[:], idx_da[:])
            idx_b = cpool.tile([P, ltot // 16 - 1024], mybir.dt.int16)
            nc.sync.dma_start(idx_b[:], idx_db[:])
            m_all = cpool.tile([P, ltot // P], mybir.dt.bfloat16)
            nc.sync.dma_start(m_all[:], m_d[:])

            gq = [0]
            g_tiles = {}
            s_tiles = {}

            def ensure(ci):
                if ci in g_tiles:
                    return
                idx_off, n_idx, base, src_rows, blk_off = calls[ci]
                nb_s = n_idx // P
                g = gpool.tile([P, NBC, D], mybir.dt.bfloat16, tag="g")
                c0 = idx_off // 16
                if c0 + n_idx // 16 <= 1024:
                    islice = idx_a[:, c0: c0 + n_idx // 16]
                else:
                    islice = idx_b[:, c0 - 1024: c0 - 1024 + n_idx // 16]
                nc.gpsimd.dma_gather(
                    g[:, :nb_s, :],
                    wt[base:base + src_rows, :],
                    islice,
                    n_idx, n_idx, D, queue_num=gq[0] % 4,
                )
                gq[0] += 1
                st = spool.tile([P, NBC * P], mybir.dt.bfloat16, tag="s")
                nc.vector.tensor_tensor(
                    out=st[:, :n_idx].rearrange("p (b m) -> p b m", m=P),
                    in0=m_all[:, blk_off: blk_off + nb_s].unsqueeze(2)
                        .broadcast_to([P, nb_s, P]),
                    in1=iota_bf[:].unsqueeze(1).broadcast_to([P, nb_s, P]),
                    op=mybir.AluOpType.is_equal,
                )
                g_tiles[ci] = g
                s_tiles[ci] = st

            b = 0
            for j in range(SLOTS):
                nbj = int(nbs[j])
                ps = pspool.tile([P, D], mybir.dt.float32, space="PSUM")
                for i in range(nbj):
                    ci = int(blk2call[b])
                    b_loc = b - calls[ci][4]
                    ensure(ci)
                    nc.tensor.matmul(
                        out=ps[:],
                        lhsT=s_tiles[ci][:, b_loc * P:(b_loc + 1) * P],
                        rhs=g_tiles[ci][:, b_loc, :],
                        start=(i == 0),
                        stop=(i == nbj - 1),
                    )
                    b += 1
                ot = opool.tile([P, D], mybir.dt.float32, tag="o")
                nc.vector.tensor_tensor(
                    out=ot[:], in0=ps[:], in1=bias_t[:],
                    op=mybir.AluOpType.add,
                )
                nc.sync.dma_start(out[j * P:(j + 1) * P, :], ot[:])
    nc.compile()
    return nc


def kernel(x=None, edge_index=None, W_weight=None, W_bias=None, _trace=False):
    global LAST_EXEC_NS
    edge_index = np.asarray(edge_index)
    W_weight = np.asarray(W_weight, dtype=np.float32)
    W_bias = np.asarray(W_bias, dtype=np.float32)

    key = (edge_index.tobytes()[:4096], edge_index.shape)
    cached = _CACHE.get(key)
    if cached is None:
        nbs, tile_map, calls, blk2call, idx_arr, m_arr = _prepare(edge_index)
        nc = _build(nbs, calls, blk2call)
        wt_bf = np.ascontiguousarray(W_weight.T).astype(ml_dtypes.bfloat16)
        bias_b = np.tile(W_bias[None, :], (P, 1)).astype(np.float32)
        in_maps = []
        for c in range(NCORE):
            im = {
                "wt": wt_bf,
                "bias": bias_b,
                "idxa": np.ascontiguousarray(
                    np.tile(idx_arr[c].reshape(-1, 16).T, (8, 1))[:, :1024]
                ).astype(np.int16),
                "idxb": np.ascontiguousarray(
                    np.tile(idx_arr[c].reshape(-1, 16).T, (8, 1))[:, 1024:]
                ).astype(np.int16),
                "m": np.ascontiguousarray(
                    m_arr[c].reshape(-1, P).T).astype(ml_dtypes.bfloat16),
            }
            in_maps.append(im)
        _CACHE[key] = (nc, in_maps, tile_map)
    else:
        nc, in_maps, tile_map = cached

    res = run_bass_kernel_spmd(nc, in_maps, core_ids=list(range(NCORE)),
                               trace=_trace)
    LAST_EXEC_NS = res.exec_time_ns
    outp = np.zeros((N, D), np.float32)
    for c in range(NCORE):
        r = res.results[c]["out"].astype(np.float32)
        for j in range(SLOTS):
            t = int(tile_map[c, j])
            if t < 0:
                continue
            rows = min(P, N - t * P)
            outp[t * P: t * P + rows] = r[j * P: j * P + rows]
    return outp

